# revision 23
# baseline (speedup 1.0000x reference)
"""Trainium2 Bass kernel for nn_DeformConv2d (DCNv3-style deformable conv).

Data-parallel over batch N=8 across 8 NeuronCores (one image per core).

Per-core pipeline (matmul/stencil tensors in CP layout [channel-on-partition,
pixel-on-free] so pixel shifts are free-dim AP offsets):
  host-prepadded bf16 x -> depthwise 3x3 (PE bf16 diag-matmuls) ->
  offset/mask matmuls in PP layout -> hats (ACT) -> A-coefficient outer
  products (DVE, g-batched 4-dim-AP scatter adds) -> A transposed to CP via
  PE identity-matmuls, compacted on-chip, written contiguously to DRAM ->
  proj_input (PE) -> 25-tap spatially-varying stencil: A rows broadcast-
  DMA'd across partitions (the DMA-queue aggregate ~220GB/s is the
  bottleneck, so the image is processed in row-halves to start streaming
  ~40us earlier and keep the queues saturated end-to-end); DVE does only
  the 50 products per half; the tap-sum accumulates on the otherwise-idle
  PE as identity-matmuls into persistent PSUM -> proj_output as 2-matmul
  PSUM groups, staged contiguously, reassembled on host.
"""

import numpy as np
import ml_dtypes

# ---- hardcoded problem constants ----
N, H, W, C = 8, 64, 64, 256
G, KS, K = 4, 3, 9
GD = C // G                     # 64
PADH = 2
Hp, Wp = H + 2 * PADH, W + 2 * PADH      # 68, 68
L = H * W                        # 4096
Lh = L // 2                      # 2048 pixels per row-half
Lp = Hp * Wp                     # 4624
NBLK = (Lp + 127) // 128         # 37
Lpb = NBLK * 128                 # 4736
GRD = 144                        # xt guard elems each side (dw halo)
FCP = GRD + Lpb + GRD            # 5024
NUB = L // 128                   # 32 unpadded output blocks
NQ = (Lpb + 511) // 512          # 10 pixel chunks (last = 128)
INTB = PADH * Wp + PADH          # 138: first interior pixel in padded coords
HBLK = 20                        # A-build blocks in first half

BLIST = (1, 3, 5, 7, 9, 13, 15, 17, 21, 23)   # taps broadcast on PE, not DMA

BF16 = ml_dtypes.bfloat16
_CACHE = {}
_TRACE = False
_LAST_EXEC_NS = None


def _host_consts(w_in, w_out, w_dw, w_pw):
    c = {}
    c["win_t"] = np.ascontiguousarray(w_in.T).astype(BF16)      # [c', c]
    c["wout_t"] = np.ascontiguousarray(w_out.T).astype(BF16)
    wpt = w_pw.T.astype(np.float32)                              # [c', 112]
    # om channel = (g*K + k)*2 + axis (x=0/y=1); mask = 72 + g*K + k
    wpc = np.concatenate([wpt[:, 0:72:2], wpt[:, 1:72:2], wpt[:, 72:108]],
                         axis=1)                                 # [c', 108]
    c["wpw_c"] = np.ascontiguousarray(wpc).astype(BF16)
    wdw = w_dw.reshape(KS * KS, C)
    dg = np.zeros((KS * KS, 2, 128, 128), np.float32)
    for t in range(KS * KS):
        for ct in range(2):
            np.fill_diagonal(dg[t, ct], wdw[t, ct * 128:(ct + 1) * 128])
    c["wdw_diag"] = dg.astype(BF16)
    c["ident"] = np.eye(128, dtype=np.float32).astype(BF16)
    sel = np.zeros((2, len(BLIST), 128, 128), np.float32)
    for ct in range(2):
        for bi, tap in enumerate(BLIST):
            for ch in range(128):
                sel[ct, bi, (2 * ct + ch // 64) * 25 + tap, ch] = 1.0
    c["sel"] = sel.astype(BF16)
    return c


def _pad_image(xn):
    """[L, C] f32 -> prepadded CP bf16 [128, 2, FCP] (zeros in guards/pads)."""
    xt = xn.T.astype(BF16)                       # [C, L]
    grid = np.zeros((128, 2, Hp, Wp), BF16)
    arr = xt.reshape(2, 128, H, W)
    grid[:, :, PADH:PADH + H, PADH:PADH + W] = arr.transpose(1, 0, 2, 3)
    full = np.zeros((128, 2, FCP), BF16)
    full[:, :, GRD:GRD + Lp] = grid.reshape(128, 2, Lp)
    return full


def _build_kernel():
    import concourse.bass as bass
    import concourse.bacc as bacc
    import concourse.tile as tile
    from concourse import mybir

    def _sub(ap, dims, off=0):
        return bass.AP(ap.tensor, ap.offset + off, [list(ap.ap[0])] + dims)

    f32 = mybir.dt.float32
    bf16 = mybir.dt.bfloat16
    Act = mybir.ActivationFunctionType

    nc = bacc.Bacc("TRN2", target_bir_lowering=False, debug=False)

    def mmr(psum, lhsT, rhs, start, stop):
        nc.tensor.matmul(psum, lhsT, rhs, start=start, stop=stop)

    xtp_d = nc.dram_tensor("xtp", [128, 2 * FCP], bf16, kind="ExternalInput").ap()
    win_d = nc.dram_tensor("win_t", [C, C], bf16, kind="ExternalInput").ap()
    wout_d = nc.dram_tensor("wout_t", [C, C], bf16, kind="ExternalInput").ap()
    wpc_d = nc.dram_tensor("wpw_c", [C, 108], bf16, kind="ExternalInput").ap()
    wdwd_d = nc.dram_tensor("wdw_diag", [KS * KS, 2, 128, 128], bf16,
                            kind="ExternalInput").ap()
    id_d = nc.dram_tensor("ident", [128, 128], bf16, kind="ExternalInput").ap()
    sel_d = nc.dram_tensor("sel", [2, len(BLIST), 128, 128], bf16,
                           kind="ExternalInput").ap()
    out_d = nc.dram_tensor("out", [128, NUB * C], bf16, kind="ExternalOutput").ap()
    # separate DRAM scratch per row-half so the tile framework orders
    # half-1 broadcasts only on the half-1 write
    at_dram = [nc.dram_tensor(f"at_scr{h}", [128, Lh], bf16).ap()
               for h in range(2)]

    with tile.TileContext(nc) as tc:
        with (
            tc.tile_pool(name="consts", bufs=1) as consts,
            tc.tile_pool(name="mid", bufs=1) as mid,
        ):
            # ---- tensors spanning phases ----
            proj_cp = mid.tile([128, 2, Lpb], bf16, tag="proj_cp")
            acc = mid.tile([128, 2, L], bf16, tag="acc")
            at_cp = mid.tile([128, Lpb], bf16, tag="at_cp")
            at_pack0 = mid.tile([128, Lh], bf16, tag="at_pack0")
            at_pack1 = mid.tile([128, Lh], bf16, tag="at_pack1")
            at_pack = [at_pack0, at_pack1]

            p2_cm = tc.tile_pool(name="p2", bufs=1)
            p2 = p2_cm.__enter__()
            ompp = p2.tile([128, NBLK, 3, 36], bf16, tag="ompp")
            abf = p2.tile([128, NBLK, 128], bf16, tag="abf")
            p1_cm = tc.tile_pool(name="p1", bufs=1)
            p1 = p1_cm.__enter__()
            xt_cp = p1.tile([128, 2, FCP], bf16, tag="xt_cp")
            nc.sync.dma_start(out=xt_cp,
                              in_=xtp_d.rearrange("p (a f) -> p a f", f=FCP))
            nc.gpsimd.memset(abf, 0)

            # ---- consts ----
            win_sb = consts.tile([128, 2, C], bf16, tag="win")
            nc.sync.dma_start(out=win_sb, in_=win_d.rearrange("(a p) c -> p a c", p=128))
            wout_sb = consts.tile([128, 2, C], bf16, tag="wout")
            nc.sync.dma_start(out=wout_sb, in_=wout_d.rearrange("(a p) c -> p a c", p=128))
            wpc_sb = consts.tile([128, 2, 108], bf16, tag="wpc")
            nc.sync.dma_start(out=wpc_sb, in_=wpc_d.rearrange("(a p) c -> p a c", p=128))
            wdw_sb = consts.tile([128, KS * KS, 2, 128], bf16, tag="wdw")
            nc.sync.dma_start(out=wdw_sb, in_=wdwd_d.rearrange("t a p c -> p t a c"))
            id_sb = consts.tile([128, 128], bf16, tag="ident")
            nc.sync.dma_start(out=id_sb, in_=id_d)
            sel_sb = consts.tile([128, 2, len(BLIST), 128], bf16, tag="sel")
            nc.sync.dma_start(out=sel_sb, in_=sel_d.rearrange("a t p c -> p a t c"))
            biasv = consts.tile([128, 3], f32, tag="biasv")
            for d in range(3):
                nc.vector.memset(biasv[:, d:d + 1], float(-(d - 1)))

            # phase-2 temporaries (allocated up front; ops emitted per half)
            habs = p2.tile([128, NBLK, 36], f32, tag="habs")
            hpp = p2.tile([128, NBLK, 2, 3, 36], bf16, tag="hpp")
            a_pp = p2.tile([128, NBLK, G, 25], f32, tag="a_pp")
            t36 = p2.tile([128, NBLK, 36], bf16, tag="t36")
            nc.vector.memset(a_pp, 0)

            def hats_and_a(blk0, blk1):
                nb = blk1 - blk0
                # hats in PP: h[ax][d] = relu(1 - |o - (d-1)|)
                hab = _sub(habs, [[36, nb], [1, 36]], blk0 * 36)
                for ax in range(2):
                    osl = _sub(ompp, [[3 * 36, nb], [1, 36]],
                               blk0 * 3 * 36 + ax * 36)
                    for d in range(3):
                        nc.scalar.activation(hab, osl, Act.Abs,
                                             bias=biasv[:, d:d + 1], scale=1.0)
                        hsl = _sub(hpp, [[2 * 3 * 36, nb], [1, 36]],
                                   blk0 * 2 * 3 * 36 + (ax * 3 + d) * 36)
                        nc.scalar.activation(hsl, hab, Act.Relu,
                                             bias=1.0, scale=-1.0)
                # fold mask into y-hats
                msl = _sub(ompp, [[3 * 36, nb], [1, 36]], blk0 * 3 * 36 + 2 * 36)
                for d in range(3):
                    hsl = _sub(hpp, [[2 * 3 * 36, nb], [1, 36]],
                               blk0 * 2 * 3 * 36 + (3 + d) * 36)
                    nc.vector.tensor_mul(hsl, hsl, msl)
                # A outer products, batched over g: per (dy,dx) one mul
                # [nb,36] + one scatter-add with a 4-dim AP
                for dy in range(3):
                    for dx in range(3):
                        in0 = _sub(hpp, [[2 * 3 * 36, nb], [1, 36]],
                                   blk0 * 2 * 3 * 36 + (3 + dy) * 36)
                        in1 = _sub(hpp, [[2 * 3 * 36, nb], [1, 36]],
                                   blk0 * 2 * 3 * 36 + dx * 36)
                        tsl = _sub(t36, [[36, nb], [1, 36]], blk0 * 36)
                        nc.vector.tensor_mul(tsl, in0, in1)
                        asl = _sub(a_pp,
                                   [[G * 25, nb], [25, G], [5, KS], [1, KS]],
                                   blk0 * G * 25 + dy * 5 + dx)
                        tsc = _sub(t36, [[36, nb], [K, G], [KS, KS], [1, KS]],
                                   blk0 * 36)
                        nc.vector.tensor_add(asl, asl, tsc)
                # cast this half's A to bf16 rows [g*25+tap]
                nc.vector.tensor_copy(
                    _sub(abf, [[128, nb], [1, 100]], blk0 * 128),
                    _sub(a_pp, [[100, nb], [1, 100]], blk0 * 100))

            p1s_cm = tc.tile_pool(name="p1s", bufs=2)
            p1s = p1s_cm.__enter__()
            psA_cm = tc.tile_pool(name="psA", bufs=2, space="PSUM")
            psA = psA_cm.__enter__()
            psB_cm = tc.tile_pool(name="psB", bufs=2, space="PSUM")
            psB = psB_cm.__enter__()

            def dw_om(q):
                w0 = q * 512
                wlen = min(512, Lpb - w0)
                dwt = p1s.tile([128, 2, 512], bf16, tag="dwt")
                for ct in range(2):
                    psum = psA.tile([128, 512], f32, tag="psdw")
                    for t in range(KS * KS):
                        ky, kx = t // KS, t % KS
                        s = (ky - 1) * Wp + (kx - 1)
                        rhs = xt_cp[:, ct, GRD + w0 + s: GRD + w0 + s + wlen]
                        nc.tensor.matmul(
                            psum[:, :wlen], wdw_sb[:, t, ct, :], rhs,
                            start=(t == 0), stop=(t == KS * KS - 1))
                    nc.scalar.copy(dwt[:, ct, :wlen], psum[:, :wlen])
                for b in range(wlen // 128):
                    blk = q * 4 + b
                    psom = psB.tile([128, 128], f32, tag="psT")
                    for ct in range(2):
                        mmr(_sub(psom, [[1, 108]]),
                            dwt[:, ct, b * 128:(b + 1) * 128],
                            wpc_sb[:, ct, :],
                            start=(ct == 0), stop=(ct == 1))
                    nc.scalar.copy(ompp[:, blk, :, :],
                        _sub(psom, [[36, 3], [1, 36]]))

            def transp(blk0, blk1):
                for blk in range(blk0, blk1):
                    psT = psB.tile([128, 128], f32, tag="psT")
                    mmr(psT, abf[:, blk, :], id_sb, start=True, stop=True)
                    nc.scalar.copy(at_cp[:, blk * 128:(blk + 1) * 128], psT)

            def compact_write(h):
                # interior rows 32h..32h+31 -> packed [128, Lh] -> DRAM
                nc.scalar.copy(
                    _sub(at_pack[h], [[W, 32], [1, W]]),
                    _sub(at_cp, [[Wp, 32], [1, W]], INTB + 32 * h * Wp))
                nc.sync.dma_start(out=at_dram[h], in_=at_pack[h])

            def proj_mc(mc):
                for q in range(NQ):
                    w0 = q * 512
                    wlen = min(512, Lpb - w0)
                    psum = psA.tile([128, 512], f32, tag="psdw")
                    for kc in range(2):
                        mmr(psum[:, :wlen],
                            win_sb[:, kc, mc * 128:(mc + 1) * 128],
                            xt_cp[:, kc, GRD + w0: GRD + w0 + wlen],
                            start=(kc == 0), stop=(kc == 1))
                    nc.scalar.copy(
                        proj_cp[:, mc, w0: w0 + wlen],
                        psum[:, :wlen])

            # ====== phase 1: half-0 A chain + write + ct0 projection
            # first so half-0 broadcasts and products start ASAP; dw q5-9
            # and the half-1 A chain follow on the then-idle engines ======
            dw_om(0)
            dw_om(1)
            dw_om(2)
            hats_and_a(0, 12)
            dw_om(3)
            dw_om(4)
            hats_and_a(12, HBLK)
            transp(0, HBLK)
            compact_write(0)
            proj_mc(0)
            dw_om(5)
            dw_om(6)
            dw_om(7)
            dw_om(8)
            dw_om(9)
            proj_mc(1)


            # ====== stencil (per row-half, per ct): DVE products, PE
            # identity-matmul accumulation into persistent PSUM ======
            p3a_cm = tc.tile_pool(name="p3a", bufs=5)
            p3a = p3a_cm.__enter__()
            p3q_cm = tc.tile_pool(name="p3q", bufs=5)
            p3q = p3q_cm.__enter__()
            p3d_cm = tc.tile_pool(name="p3d", bufs=2)
            p3d = p3d_cm.__enter__()
            psP_cm = tc.tile_pool(name="psP", bufs=1, space="PSUM")
            psP = psP_cm.__enter__()

            BSET = frozenset(BLIST)

            DSET = (6, 12, 18, 24)               # taps accumulated on DVE

            def stencil(h, ct, weave=None):
                pacc = psP.tile([128, Lh], f32, tag="pacc")
                acc_dve = p3d.tile([128, Lh], bf16, tag="acc_dve")
                npe = 0
                nd = 0
                for ty in range(5):
                    for tx in range(5):
                        tap = ty * 5 + tx
                        s = (ty - 2) * Wp + (tx - 2)
                        aexp = p3a.tile([128, Lh], bf16, tag="aexp")
                        if tap in BSET:
                            # broadcast A rows on the PE instead of DMA:
                            # aexp[c,:] = sel_t^T @ at_pack
                            for ch in range(Lh // 512):
                                psb = psA.tile([128, 512], f32, tag="psdw")
                                mmr(psb, sel_sb[:, ct, BLIST.index(tap), :],
                                    at_pack[h][:, ch * 512:(ch + 1) * 512],
                                    start=True, stop=True)
                                nc.scalar.copy(
                                    aexp[:, ch * 512:(ch + 1) * 512], psb)
                        else:
                            for gh in range(2):
                                row = (2 * ct + gh) * 25 + tap
                                nc.sync.dma_start(
                                    out=aexp[gh * 64:(gh + 1) * 64, :],
                                    in_=bass.AP(at_dram[h].tensor,
                                                at_dram[h].offset + row * Lh,
                                                [[0, 64], [1, Lh]]))
                        src_v = _sub(proj_cp, [[Wp, 32], [1, W]],
                                     ct * Lpb + INTB + 32 * h * Wp + s)
                        if tap in DSET:
                            # DVE-accumulated tap: product straight into
                            # the SBUF side-accumulator
                            if nd == 0:
                                nc.vector.tensor_mul(
                                    _sub(acc_dve, [[W, 32], [1, W]]),
                                    src_v, _sub(aexp, [[W, 32], [1, W]]))
                            else:
                                qt = p3q.tile([128, Lh], bf16, tag="qt")
                                nc.vector.tensor_mul(
                                    _sub(qt, [[W, 32], [1, W]]),
                                    src_v, _sub(aexp, [[W, 32], [1, W]]))
                                nc.vector.tensor_add(acc_dve, acc_dve, qt)
                            nd += 1
                        else:
                            qt = p3q.tile([128, Lh], bf16, tag="qt")
                            nc.vector.tensor_mul(
                                _sub(qt, [[W, 32], [1, W]]),
                                src_v, _sub(aexp, [[W, 32], [1, W]]))
                            for ch in range(Lh // 512):
                                mmr(pacc[:, ch * 512:(ch + 1) * 512], id_sb,
                                    qt[:, ch * 512:(ch + 1) * 512],
                                    start=(npe == 0), stop=False)
                            npe += 1
                # fold the DVE accumulator into PSUM and close the groups
                for ch in range(Lh // 512):
                    mmr(pacc[:, ch * 512:(ch + 1) * 512], id_sb,
                        acc_dve[:, ch * 512:(ch + 1) * 512],
                        start=False, stop=True)
                nc.scalar.copy(
                    _sub(acc, [[1, Lh]], ct * L + h * Lh), pacc)

            out_sb = mid.tile([128, NUB, C], bf16, tag="out_sb")

            def proj_out_half(h):
                # acc rows of half h are final: project + stage + store;
                # the DRAM write streams out per 4 blocks
                for ub in range(16 * h, 16 * h + 16):
                    psum = psA.tile([128, 512], f32, tag="psdw")
                    pso = _sub(psum, [[1, C]])
                    for ct in range(2):
                        mmr(pso, acc[:, ct, ub * 128:(ub + 1) * 128],
                            wout_sb[:, ct, :], start=(ct == 0), stop=(ct == 1))
                    nc.scalar.copy(out_sb[:, ub, :], pso)
                    if ub % 4 == 3:
                        nc.sync.dma_start(
                            out=out_d[:, (ub - 3) * C:(ub + 1) * C],
                            in_=_sub(out_sb, [[1, 4 * C]], (ub - 3) * C))

            # second-half A chain still in the head (DVE/ACT), then the
            # pure stencil phases
            hats_and_a(HBLK, NBLK)
            transp(HBLK, NBLK)
            compact_write(1)
            stencil(0, 0)
            stencil(0, 1)
            proj_out_half(0)
            stencil(1, 0)
            stencil(1, 1)
            proj_out_half(1)

            psP_cm.__exit__(None, None, None)
            p3d_cm.__exit__(None, None, None)
            p3q_cm.__exit__(None, None, None)
            p3a_cm.__exit__(None, None, None)
            psB_cm.__exit__(None, None, None)
            psA_cm.__exit__(None, None, None)
            p1s_cm.__exit__(None, None, None)
            p1_cm.__exit__(None, None, None)
            p2_cm.__exit__(None, None, None)


    nc.compile()
    return nc


def _get_compiled():
    if "nc" not in _CACHE:
        _CACHE["nc"] = _build_kernel()
    return _CACHE["nc"]


def kernel(**inputs):
    from concourse.bass_utils import run_bass_kernel_spmd

    x = np.asarray(inputs["x"], np.float32)
    for bn in ("b_in", "b_out", "b_dw", "b_pw"):
        assert not np.any(np.asarray(inputs[bn])), f"nonzero bias {bn} unsupported"
    consts = _host_consts(
        np.asarray(inputs["w_in"], np.float32),
        np.asarray(inputs["w_out"], np.float32),
        np.asarray(inputs["w_dw"], np.float32),
        np.asarray(inputs["w_pw"], np.float32))

    nc = _get_compiled()
    in_maps = []
    for n in range(N):
        m = {"xtp": _pad_image(x[n]).reshape(128, 2 * FCP)}
        m.update(consts)
        in_maps.append(m)

    global _LAST_EXEC_NS
    res = run_bass_kernel_spmd(nc, in_maps, list(range(N)), trace=_TRACE)
    _LAST_EXEC_NS = res.exec_time_ns
    out = np.stack([
        np.asarray(res.results[i]["out"])
        .reshape(128, NUB, C).transpose(1, 0, 2).reshape(L, C)
        for i in range(N)
    ])
    return out.astype(np.float32)


# revision 24
# speedup vs baseline: 1.0332x; 1.0332x over previous
"""Trainium2 Bass kernel for nn_DeformConv2d (DCNv3-style deformable conv).

Data-parallel over batch N=8 across 8 NeuronCores (one image per core).

Per-core pipeline (matmul/stencil tensors in CP layout [channel-on-partition,
pixel-on-free] so pixel shifts are free-dim AP offsets):
  host-prepadded bf16 x -> depthwise 3x3 (PE bf16 diag-matmuls) ->
  offset/mask matmuls in PP layout -> hats (ACT) -> A-coefficient outer
  products (DVE, g-batched 4-dim-AP scatter adds) -> A transposed to CP via
  PE identity-matmuls, compacted on-chip, written contiguously to DRAM ->
  proj_input (PE) -> 25-tap spatially-varying stencil: A rows broadcast-
  DMA'd across partitions (the DMA-queue aggregate ~220GB/s is the
  bottleneck, so the image is processed in row-halves to start streaming
  ~40us earlier and keep the queues saturated end-to-end); DVE does only
  the 50 products per half; the tap-sum accumulates on the otherwise-idle
  PE as identity-matmuls into persistent PSUM -> proj_output as 2-matmul
  PSUM groups, staged contiguously, reassembled on host.
"""

import numpy as np
import ml_dtypes

# ---- hardcoded problem constants ----
N, H, W, C = 8, 64, 64, 256
G, KS, K = 4, 3, 9
GD = C // G                     # 64
PADH = 2
Hp, Wp = H + 2 * PADH, W + 2 * PADH      # 68, 68
L = H * W                        # 4096
Lh = L // 2                      # 2048 pixels per row-half
Lp = Hp * Wp                     # 4624
NBLK = (Lp + 127) // 128         # 37
Lpb = NBLK * 128                 # 4736
GRD = 144                        # xt guard elems each side (dw halo)
FCP = GRD + Lpb + GRD            # 5024
NUB = L // 128                   # 32 unpadded output blocks
NQ = (Lpb + 511) // 512          # 10 pixel chunks (last = 128)
INTB = PADH * Wp + PADH          # 138: first interior pixel in padded coords
HBLK = 20                        # A-build blocks in first half

BLIST = (1, 3, 5, 7, 9, 13, 15, 17, 21, 23)   # taps broadcast on PE, not DMA

BF16 = ml_dtypes.bfloat16
_CACHE = {}
_TRACE = False
_LAST_EXEC_NS = None


def _host_consts(w_in, w_out, w_dw, w_pw):
    c = {}
    c["win_t"] = np.ascontiguousarray(w_in.T).astype(BF16)      # [c', c]
    c["wout_t"] = np.ascontiguousarray(w_out.T).astype(BF16)
    wpt = w_pw.T.astype(np.float32)                              # [c', 112]
    # om channel = (g*K + k)*2 + axis (x=0/y=1); mask = 72 + g*K + k
    wpc = np.concatenate([wpt[:, 0:72:2], wpt[:, 1:72:2], wpt[:, 72:108]],
                         axis=1)                                 # [c', 108]
    c["wpw_c"] = np.ascontiguousarray(wpc).astype(BF16)
    wdw = w_dw.reshape(KS * KS, C)
    dg = np.zeros((KS * KS, 2, 128, 128), np.float32)
    for t in range(KS * KS):
        for ct in range(2):
            np.fill_diagonal(dg[t, ct], wdw[t, ct * 128:(ct + 1) * 128])
    c["wdw_diag"] = dg.astype(BF16)
    c["ident"] = np.eye(128, dtype=np.float32).astype(BF16)
    sel = np.zeros((2, len(BLIST), 128, 128), np.float32)
    for ct in range(2):
        for bi, tap in enumerate(BLIST):
            for ch in range(128):
                sel[ct, bi, (2 * ct + ch // 64) * 25 + tap, ch] = 1.0
    c["sel"] = sel.astype(BF16)
    return c


def _pad_image(xn):
    """[L, C] f32 -> prepadded CP bf16 [128, 2, FCP] (zeros in guards/pads)."""
    xt = xn.T.astype(BF16)                       # [C, L]
    grid = np.zeros((128, 2, Hp, Wp), BF16)
    arr = xt.reshape(2, 128, H, W)
    grid[:, :, PADH:PADH + H, PADH:PADH + W] = arr.transpose(1, 0, 2, 3)
    full = np.zeros((128, 2, FCP), BF16)
    full[:, :, GRD:GRD + Lp] = grid.reshape(128, 2, Lp)
    return full


def _build_kernel():
    import concourse.bass as bass
    import concourse.bacc as bacc
    import concourse.tile as tile
    from concourse import mybir

    def _sub(ap, dims, off=0):
        return bass.AP(ap.tensor, ap.offset + off, [list(ap.ap[0])] + dims)

    f32 = mybir.dt.float32
    bf16 = mybir.dt.bfloat16
    Act = mybir.ActivationFunctionType

    nc = bacc.Bacc("TRN2", target_bir_lowering=False, debug=False)

    def mmr(psum, lhsT, rhs, start, stop):
        nc.tensor.matmul(psum, lhsT, rhs, start=start, stop=stop)

    xtp_d = nc.dram_tensor("xtp", [128, 2 * FCP], bf16, kind="ExternalInput").ap()
    win_d = nc.dram_tensor("win_t", [C, C], bf16, kind="ExternalInput").ap()
    wout_d = nc.dram_tensor("wout_t", [C, C], bf16, kind="ExternalInput").ap()
    wpc_d = nc.dram_tensor("wpw_c", [C, 108], bf16, kind="ExternalInput").ap()
    wdwd_d = nc.dram_tensor("wdw_diag", [KS * KS, 2, 128, 128], bf16,
                            kind="ExternalInput").ap()
    id_d = nc.dram_tensor("ident", [128, 128], bf16, kind="ExternalInput").ap()
    sel_d = nc.dram_tensor("sel", [2, len(BLIST), 128, 128], bf16,
                           kind="ExternalInput").ap()
    out_d = nc.dram_tensor("out", [128, NUB * C], bf16, kind="ExternalOutput").ap()
    # separate DRAM scratch per row-half so the tile framework orders
    # half-1 broadcasts only on the half-1 write
    at_dram = [nc.dram_tensor(f"at_scr{h}", [128, Lh], bf16).ap()
               for h in range(2)]

    with tile.TileContext(nc) as tc:
        with (
            tc.tile_pool(name="consts", bufs=1) as consts,
            tc.tile_pool(name="mid", bufs=1) as mid,
        ):
            # ---- tensors spanning phases ----
            proj_cp = mid.tile([128, 2, Lpb], bf16, tag="proj_cp")
            acc = mid.tile([128, 2, L], bf16, tag="acc")
            at_cp = mid.tile([128, Lpb], bf16, tag="at_cp")
            at_pack0 = mid.tile([128, Lh], bf16, tag="at_pack0")
            at_pack1 = mid.tile([128, Lh], bf16, tag="at_pack1")
            at_pack = [at_pack0, at_pack1]

            p2_cm = tc.tile_pool(name="p2", bufs=1)
            p2 = p2_cm.__enter__()
            ompp = p2.tile([128, NBLK, 3, 36], bf16, tag="ompp")
            abf = p2.tile([128, NBLK, 128], bf16, tag="abf")
            p1_cm = tc.tile_pool(name="p1", bufs=1)
            p1 = p1_cm.__enter__()
            xt_cp = p1.tile([128, 2, FCP], bf16, tag="xt_cp")
            nc.sync.dma_start(out=xt_cp,
                              in_=xtp_d.rearrange("p (a f) -> p a f", f=FCP))
            nc.gpsimd.memset(abf, 0)

            # ---- consts ----
            win_sb = consts.tile([128, 2, C], bf16, tag="win")
            nc.sync.dma_start(out=win_sb, in_=win_d.rearrange("(a p) c -> p a c", p=128))
            wout_sb = consts.tile([128, 2, C], bf16, tag="wout")
            nc.sync.dma_start(out=wout_sb, in_=wout_d.rearrange("(a p) c -> p a c", p=128))
            wpc_sb = consts.tile([128, 2, 108], bf16, tag="wpc")
            nc.sync.dma_start(out=wpc_sb, in_=wpc_d.rearrange("(a p) c -> p a c", p=128))
            wdw_sb = consts.tile([128, KS * KS, 2, 128], bf16, tag="wdw")
            nc.sync.dma_start(out=wdw_sb, in_=wdwd_d.rearrange("t a p c -> p t a c"))
            id_sb = consts.tile([128, 128], bf16, tag="ident")
            nc.sync.dma_start(out=id_sb, in_=id_d)
            sel_sb = consts.tile([128, 2, len(BLIST), 128], bf16, tag="sel")
            nc.sync.dma_start(out=sel_sb, in_=sel_d.rearrange("a t p c -> p a t c"))
            biasv = consts.tile([128, 3], f32, tag="biasv")
            for d in range(3):
                nc.vector.memset(biasv[:, d:d + 1], float(-(d - 1)))

            # phase-2 temporaries (allocated up front; ops emitted per half)
            habs = p2.tile([128, NBLK, 36], f32, tag="habs")
            hpp = p2.tile([128, NBLK, 2, 3, 36], bf16, tag="hpp")
            a_pp = p2.tile([128, NBLK, G, 25], f32, tag="a_pp")
            t36 = p2.tile([128, NBLK, 36], bf16, tag="t36")
            nc.vector.memset(a_pp, 0)

            def hats_and_a(blk0, blk1):
                nb = blk1 - blk0
                # hats in PP: h[ax][d] = relu(1 - |o - (d-1)|)
                hab = _sub(habs, [[36, nb], [1, 36]], blk0 * 36)
                for ax in range(2):
                    osl = _sub(ompp, [[3 * 36, nb], [1, 36]],
                               blk0 * 3 * 36 + ax * 36)
                    for d in range(3):
                        nc.scalar.activation(hab, osl, Act.Abs,
                                             bias=biasv[:, d:d + 1], scale=1.0)
                        hsl = _sub(hpp, [[2 * 3 * 36, nb], [1, 36]],
                                   blk0 * 2 * 3 * 36 + (ax * 3 + d) * 36)
                        nc.scalar.activation(hsl, hab, Act.Relu,
                                             bias=1.0, scale=-1.0)
                # fold mask into y-hats
                msl = _sub(ompp, [[3 * 36, nb], [1, 36]], blk0 * 3 * 36 + 2 * 36)
                for d in range(3):
                    hsl = _sub(hpp, [[2 * 3 * 36, nb], [1, 36]],
                               blk0 * 2 * 3 * 36 + (3 + d) * 36)
                    nc.vector.tensor_mul(hsl, hsl, msl)
                # A outer products, batched over g: per (dy,dx) one mul
                # [nb,36] + one scatter-add with a 4-dim AP
                for dy in range(3):
                    for dx in range(3):
                        in0 = _sub(hpp, [[2 * 3 * 36, nb], [1, 36]],
                                   blk0 * 2 * 3 * 36 + (3 + dy) * 36)
                        in1 = _sub(hpp, [[2 * 3 * 36, nb], [1, 36]],
                                   blk0 * 2 * 3 * 36 + dx * 36)
                        tsl = _sub(t36, [[36, nb], [1, 36]], blk0 * 36)
                        nc.vector.tensor_mul(tsl, in0, in1)
                        asl = _sub(a_pp,
                                   [[G * 25, nb], [25, G], [5, KS], [1, KS]],
                                   blk0 * G * 25 + dy * 5 + dx)
                        tsc = _sub(t36, [[36, nb], [K, G], [KS, KS], [1, KS]],
                                   blk0 * 36)
                        nc.vector.tensor_add(asl, asl, tsc)
                # cast this half's A to bf16 rows [g*25+tap]
                nc.vector.tensor_copy(
                    _sub(abf, [[128, nb], [1, 100]], blk0 * 128),
                    _sub(a_pp, [[100, nb], [1, 100]], blk0 * 100))

            p1s_cm = tc.tile_pool(name="p1s", bufs=2)
            p1s = p1s_cm.__enter__()
            psA_cm = tc.tile_pool(name="psA", bufs=2, space="PSUM")
            psA = psA_cm.__enter__()
            psB_cm = tc.tile_pool(name="psB", bufs=2, space="PSUM")
            psB = psB_cm.__enter__()

            def dw_om(q):
                w0 = q * 512
                wlen = min(512, Lpb - w0)
                dwt = p1s.tile([128, 2, 512], bf16, tag="dwt")
                for ct in range(2):
                    psum = psA.tile([128, 512], f32, tag="psdw")
                    for t in range(KS * KS):
                        ky, kx = t // KS, t % KS
                        s = (ky - 1) * Wp + (kx - 1)
                        rhs = xt_cp[:, ct, GRD + w0 + s: GRD + w0 + s + wlen]
                        nc.tensor.matmul(
                            psum[:, :wlen], wdw_sb[:, t, ct, :], rhs,
                            start=(t == 0), stop=(t == KS * KS - 1))
                    nc.scalar.copy(dwt[:, ct, :wlen], psum[:, :wlen])
                for b in range(wlen // 128):
                    blk = q * 4 + b
                    psom = psB.tile([128, 128], f32, tag="psT")
                    for ct in range(2):
                        mmr(_sub(psom, [[1, 108]]),
                            dwt[:, ct, b * 128:(b + 1) * 128],
                            wpc_sb[:, ct, :],
                            start=(ct == 0), stop=(ct == 1))
                    nc.scalar.copy(ompp[:, blk, :, :],
                        _sub(psom, [[36, 3], [1, 36]]))

            def transp(blk0, blk1):
                for blk in range(blk0, blk1):
                    psT = psB.tile([128, 128], f32, tag="psT")
                    mmr(psT, abf[:, blk, :], id_sb, start=True, stop=True)
                    nc.scalar.copy(at_cp[:, blk * 128:(blk + 1) * 128], psT)

            def compact_write(h):
                # interior rows 32h..32h+31 -> packed [128, Lh] -> DRAM
                nc.scalar.copy(
                    _sub(at_pack[h], [[W, 32], [1, W]]),
                    _sub(at_cp, [[Wp, 32], [1, W]], INTB + 32 * h * Wp))
                nc.sync.dma_start(out=at_dram[h], in_=at_pack[h])

            def proj_mc(mc):
                for q in range(NQ):
                    w0 = q * 512
                    wlen = min(512, Lpb - w0)
                    psum = psA.tile([128, 512], f32, tag="psdw")
                    for kc in range(2):
                        mmr(psum[:, :wlen],
                            win_sb[:, kc, mc * 128:(mc + 1) * 128],
                            xt_cp[:, kc, GRD + w0: GRD + w0 + wlen],
                            start=(kc == 0), stop=(kc == 1))
                    nc.scalar.copy(
                        proj_cp[:, mc, w0: w0 + wlen],
                        psum[:, :wlen])

            # ====== phase 1: half-0 A chain + write + ct0 projection
            # first so half-0 broadcasts and products start ASAP; dw q5-9
            # and the half-1 A chain follow on the then-idle engines ======
            dw_om(0)
            dw_om(1)
            dw_om(2)
            hats_and_a(0, 12)
            dw_om(3)
            dw_om(4)
            hats_and_a(12, HBLK)
            transp(0, HBLK)
            compact_write(0)
            proj_mc(0)
            dw_om(5)
            dw_om(6)
            dw_om(7)
            dw_om(8)
            dw_om(9)
            proj_mc(1)


            # ====== stencil (per row-half, per ct): DVE products, PE
            # identity-matmul accumulation into persistent PSUM ======
            p3a_cm = tc.tile_pool(name="p3a", bufs=6)
            p3a = p3a_cm.__enter__()
            p3q_cm = tc.tile_pool(name="p3q", bufs=4)
            p3q = p3q_cm.__enter__()
            p3d_cm = tc.tile_pool(name="p3d", bufs=2)
            p3d = p3d_cm.__enter__()
            psP_cm = tc.tile_pool(name="psP", bufs=1, space="PSUM")
            psP = psP_cm.__enter__()

            BSET = frozenset(BLIST)

            DSET = (6, 12, 18, 24)               # taps accumulated on DVE

            def stencil(h, ct, weave=None):
                pacc = psP.tile([128, Lh], f32, tag="pacc")
                acc_dve = p3d.tile([128, Lh], bf16, tag="acc_dve")
                npe = 0
                nd = 0
                for ty in range(5):
                    for tx in range(5):
                        tap = ty * 5 + tx
                        s = (ty - 2) * Wp + (tx - 2)
                        aexp = p3a.tile([128, Lh], bf16, tag="aexp")
                        if tap in BSET:
                            # broadcast A rows on the PE instead of DMA:
                            # aexp[c,:] = sel_t^T @ at_pack
                            for ch in range(Lh // 512):
                                psb = psA.tile([128, 512], f32, tag="psdw")
                                mmr(psb, sel_sb[:, ct, BLIST.index(tap), :],
                                    at_pack[h][:, ch * 512:(ch + 1) * 512],
                                    start=True, stop=True)
                                nc.scalar.copy(
                                    aexp[:, ch * 512:(ch + 1) * 512], psb)
                        else:
                            for gh in range(2):
                                row = (2 * ct + gh) * 25 + tap
                                nc.sync.dma_start(
                                    out=aexp[gh * 64:(gh + 1) * 64, :],
                                    in_=bass.AP(at_dram[h].tensor,
                                                at_dram[h].offset + row * Lh,
                                                [[0, 64], [1, Lh]]))
                        src_v = _sub(proj_cp, [[Wp, 32], [1, W]],
                                     ct * Lpb + INTB + 32 * h * Wp + s)
                        if tap in DSET:
                            # DVE-accumulated tap: product straight into
                            # the SBUF side-accumulator
                            if nd == 0:
                                nc.vector.tensor_mul(
                                    _sub(acc_dve, [[W, 32], [1, W]]),
                                    src_v, _sub(aexp, [[W, 32], [1, W]]))
                            else:
                                qt = p3q.tile([128, Lh], bf16, tag="qt")
                                nc.vector.tensor_mul(
                                    _sub(qt, [[W, 32], [1, W]]),
                                    src_v, _sub(aexp, [[W, 32], [1, W]]))
                                nc.vector.tensor_add(acc_dve, acc_dve, qt)
                            nd += 1
                        else:
                            qt = p3q.tile([128, Lh], bf16, tag="qt")
                            nc.vector.tensor_mul(
                                _sub(qt, [[W, 32], [1, W]]),
                                src_v, _sub(aexp, [[W, 32], [1, W]]))
                            for ch in range(Lh // 512):
                                mmr(pacc[:, ch * 512:(ch + 1) * 512], id_sb,
                                    qt[:, ch * 512:(ch + 1) * 512],
                                    start=(npe == 0), stop=False)
                            npe += 1
                # fold the DVE accumulator into PSUM and close the groups
                for ch in range(Lh // 512):
                    mmr(pacc[:, ch * 512:(ch + 1) * 512], id_sb,
                        acc_dve[:, ch * 512:(ch + 1) * 512],
                        start=False, stop=True)
                nc.scalar.copy(
                    _sub(acc, [[1, Lh]], ct * L + h * Lh), pacc)

            out_sb = mid.tile([128, NUB, C], bf16, tag="out_sb")

            def proj_out_half(h):
                # acc rows of half h are final: project + stage + store;
                # the DRAM write streams out per 4 blocks
                for ub in range(16 * h, 16 * h + 16):
                    psum = psA.tile([128, 512], f32, tag="psdw")
                    pso = _sub(psum, [[1, C]])
                    for ct in range(2):
                        mmr(pso, acc[:, ct, ub * 128:(ub + 1) * 128],
                            wout_sb[:, ct, :], start=(ct == 0), stop=(ct == 1))
                    nc.scalar.copy(out_sb[:, ub, :], pso)
                    if ub % 4 == 3:
                        nc.sync.dma_start(
                            out=out_d[:, (ub - 3) * C:(ub + 1) * C],
                            in_=_sub(out_sb, [[1, 4 * C]], (ub - 3) * C))

            # second-half A chain still in the head (DVE/ACT), then the
            # pure stencil phases
            hats_and_a(HBLK, NBLK)
            transp(HBLK, NBLK)
            compact_write(1)
            stencil(0, 0)
            stencil(0, 1)
            proj_out_half(0)
            stencil(1, 0)
            stencil(1, 1)
            proj_out_half(1)

            psP_cm.__exit__(None, None, None)
            p3d_cm.__exit__(None, None, None)
            p3q_cm.__exit__(None, None, None)
            p3a_cm.__exit__(None, None, None)
            psB_cm.__exit__(None, None, None)
            psA_cm.__exit__(None, None, None)
            p1s_cm.__exit__(None, None, None)
            p1_cm.__exit__(None, None, None)
            p2_cm.__exit__(None, None, None)


    nc.compile()
    return nc


def _get_compiled():
    if "nc" not in _CACHE:
        _CACHE["nc"] = _build_kernel()
    return _CACHE["nc"]


def kernel(**inputs):
    from concourse.bass_utils import run_bass_kernel_spmd

    x = np.asarray(inputs["x"], np.float32)
    for bn in ("b_in", "b_out", "b_dw", "b_pw"):
        assert not np.any(np.asarray(inputs[bn])), f"nonzero bias {bn} unsupported"
    consts = _host_consts(
        np.asarray(inputs["w_in"], np.float32),
        np.asarray(inputs["w_out"], np.float32),
        np.asarray(inputs["w_dw"], np.float32),
        np.asarray(inputs["w_pw"], np.float32))

    nc = _get_compiled()
    in_maps = []
    for n in range(N):
        m = {"xtp": _pad_image(x[n]).reshape(128, 2 * FCP)}
        m.update(consts)
        in_maps.append(m)

    global _LAST_EXEC_NS
    res = run_bass_kernel_spmd(nc, in_maps, list(range(N)), trace=_TRACE)
    _LAST_EXEC_NS = res.exec_time_ns
    out = np.stack([
        np.asarray(res.results[i]["out"])
        .reshape(128, NUB, C).transpose(1, 0, 2).reshape(L, C)
        for i in range(N)
    ])
    return out.astype(np.float32)
